# revision 3
# baseline (speedup 1.0000x reference)
"""Trainium2 Bass kernel v2 for depthwise morphological dilation (max-plus).

Same math/layout as kernel.py baseline; scheduling changes:
  - dual-ring DMA: head (block-0 x) split across qSync + qScalar; per-di
    output DMAs alternate rings; final output split across both.
  - redundant monotone sem-waits pruned before legalization (the baseline
    carried ~75 EventSemaphore instrs on ScalarE ~ 20us of issue time).
  - DVE emission interleaves the two di chains per block so DVE never
    idles waiting on ACT tmp production.
"""

import os
import sys

if "jax" not in sys.modules and os.environ.get("JAX_PLATFORMS") == "cpu":
    os.environ["JAX_PLATFORMS"] = ""

for _p in ("/opt/trn_rl_repo",):
    if _p not in sys.path:
        sys.path.insert(0, _p)

import ml_dtypes
import numpy as np

from concourse import mybir
import concourse.bass as bass
import concourse.tile as tile
from concourse.bass_utils import run_bass_kernel_spmd

BF16 = ml_dtypes.bfloat16

B, H, W, C = 4, 256, 256, 64
KH, KW, DM = 3, 3, 2
N_CORES = 8
PAD_VAL_F16 = np.float32(-60000.0)

G = 2
HG = 64
WP = W + 2
XROWS = HG + 2

KERNEL_DT = os.environ.get("KERNEL_DT", "f16")

# per-(block,di) tap assignment: init + 2 adds on DVE (all j-even, keeps
# 4x tensor_scalar), 6 adds on ACT (includes all j=1 taps).
INIT_TAP = (1, 0)
ACT_TAPS = [(0, 1), (1, 1), (2, 1), (0, 0), (2, 2), (0, 2)]
DVE_TAPS = [(2, 0), (1, 2)]
# the final (block,di) chain runs one more tap on DVE so its tail isn't
# gated on ScalarE's very last ACTIVATE
ACT_TAPS_LAST = [(0, 1), (1, 1), (2, 1), (0, 0), (2, 2)]
DVE_TAPS_LAST = [(2, 0), (1, 2), (0, 2)]

BLOCKS = [(0, 32), (32, 32)]

_CACHED = {}


def _prune_redundant_waits(nc):
    """Drop sem-ge-imm waits already implied by an earlier wait (same or
    higher threshold, same semaphore) on the same engine queue. Only for
    semaphores that are exclusively inc-updated (Tile's dependency
    counters); barrier sems (dec'd / eq-waited) are left alone."""
    unsafe = set()
    for fn in nc.m.functions:
        for blk in fn.blocks:
            for ins in blk.instructions:
                si = ins.sync_info
                if not si:
                    continue
                for u in si.on_update or []:
                    if u.update_mode != "sem-inc":
                        unsafe.add(u.id)
                for w in si.on_wait or []:
                    if w.wait_mode != "sem-ge-imm" or w.wait_reg is not None:
                        unsafe.add(w.id)
    n_drop = 0
    for fn in nc.m.functions:
        for blk in fn.blocks:
            best = {}  # (engine, sem_id) -> max threshold enforced
            for ins in blk.instructions:
                si = ins.sync_info
                if not si or not si.on_wait:
                    continue
                eng = str(ins.engine)
                keep = []
                for w in si.on_wait:
                    if (
                        w.wait_mode == "sem-ge-imm"
                        and w.wait_reg is None
                        and w.id not in unsafe
                    ):
                        k = (eng, w.id)
                        if best.get(k, -1) >= w.wait_value:
                            n_drop += 1
                            continue
                        best[k] = w.wait_value
                    keep.append(w)
                si.on_wait = keep
    return n_drop


def _legalize_waits(nc):
    """Split multi-wait instructions (TPB ISA: one sem-wait per regular
    instruction; EventSemaphore holds 2)."""
    n_extra = 0
    for fn in nc.m.functions:
        for blk in fn.blocks:
            insts = blk.instructions
            new_list = []
            for ins in insts:
                si = ins.sync_info
                waits = list(si.on_wait) if (si and si.on_wait) else []
                if len(waits) > 1 and ins.opcode != "EventSemaphore":
                    keep, extra = waits[-1], waits[:-1]
                    for k in range(0, len(extra), 2):
                        es = mybir.InstEventSemaphore(
                            name=f"legalw_{ins.name}_{k}", ins=[], outs=[]
                        )
                        es.engine = ins.engine
                        es.sync_info = mybir.SyncInfo(
                            on_wait=extra[k : k + 2], on_update=[]
                        )
                        new_list.append(es)
                        n_extra += 1
                    si.on_wait = [keep]
                new_list.append(ins)
            insts[:] = new_list
    return n_extra


def _build_bass(dt_name: str):
    key = dt_name
    if key in _CACHED:
        return _CACHED[key]
    dt = {
        "bf16": mybir.dt.bfloat16,
        "f16": mybir.dt.float16,
        "f32": mybir.dt.float32,
    }[dt_name]

    nc = bass.Bass(
        "TRN2",
        target_bir_lowering=False,
        debug=False,
        num_devices=N_CORES,
    )
    x_ap = nc.dram_tensor("x", [128, XROWS, WP], dt, kind="ExternalInput").ap()
    kt_ap = nc.dram_tensor(
        "ktab", [128, KH * KW * DM], mybir.dt.float32, kind="ExternalInput"
    ).ap()
    o_aps = [
        nc.dram_tensor(f"o{di}", [128, HG, W], dt, kind="ExternalOutput").ap()
        for di in range(DM)
    ]

    from contextlib import ExitStack

    with tile.TileContext(nc) as tc, ExitStack() as ctx:
        kpool = ctx.enter_context(tc.tile_pool(name="kp", bufs=1))
        xpool = ctx.enter_context(tc.tile_pool(name="xp", bufs=2))
        tpool = ctx.enter_context(tc.tile_pool(name="tp", bufs=6))       # ACT tmps
        vpool = ctx.enter_context(tc.tile_pool(name="vp", bufs=2))       # DVE tmps
        apool = ctx.enter_context(tc.tile_pool(name="acp", bufs=2))

        ktile = kpool.tile([128, KH * KW * DM], mybir.dt.float32, name="ktile")
        nc.sync.dma_start(ktile[:], kt_ap[:])

        # input blocks spread across both DMA rings: the tiny block-0 lands
        # first on qSync so compute starts early; block-1 rides qScalar in
        # parallel (ScalarE is idle at the head)
        xbs = []
        for bi, (r0, rows) in enumerate(BLOCKS):
            xb = xpool.tile([128, rows + 2, WP], dt, name="xb", tag="xb")
            if bi == 0:
                half = (rows + 2) // 2
                nc.sync.dma_start(xb[:, :half, :], x_ap[:, r0 : r0 + half, :])
                nc.scalar.dma_start(
                    xb[:, half : rows + 2, :],
                    x_ap[:, r0 + half : r0 + rows + 2, :],
                )
            else:
                nc.sync.dma_start(xb[:], x_ap[:, r0 : r0 + rows + 2, :])
            xbs.append(xb)

        # ACT warmup: zero-dependency ACTIVATE so the activation-table load
        # overlaps the input DMA.
        warm = kpool.tile([128, 1], dt, name="warm")
        nc.gpsimd.memset(warm[:], 0.0)
        nc.scalar.add(warm[:], warm[:], 0.0)

        def kvec(i, j, di):
            t = di * 9 + i * 3 + j
            return ktile[:, t : t + 1]

        n_blocks = len(BLOCKS)
        for blk, (r0, rows) in enumerate(BLOCKS):
            xb = xbs[blk]
            last_block = blk == n_blocks - 1

            def src(i, j):
                return xb[:, i : i + rows, j : j + W]

            # ---- ACT: tmps for this block (di0 then di1) ----
            atmps = {}
            for di in range(DM):
                final = last_block and di == DM - 1
                for i, j in ACT_TAPS_LAST if final else ACT_TAPS:
                    tmpa = tpool.tile([128, rows, W], dt, name="tmpa", tag="tmp")
                    nc.scalar.add(tmpa[:], src(i, j), kvec(i, j, di))
                    atmps[(di, i, j)] = tmpa

            # ---- DVE: per-di front (init + own adds + own maxes) ----
            accs = {}
            for di in range(DM):
                final = last_block and di == DM - 1
                acc = apool.tile([128, rows, W], dt, name="acc", tag="acc")
                i, j = INIT_TAP
                nc.vector.tensor_scalar_add(acc[:], src(i, j), kvec(i, j, di))
                accs[di] = acc
                dtmps = []
                for i, j in DVE_TAPS_LAST if final else DVE_TAPS:
                    tv = vpool.tile([128, rows, W], dt, name="tmpv", tag="vtmp")
                    nc.vector.tensor_scalar_add(tv[:], src(i, j), kvec(i, j, di))
                    dtmps.append(tv)
                for tv in dtmps:
                    nc.vector.tensor_max(acc[:], acc[:], tv[:])
            # merge ACT tmps in production order per di
            for di in range(DM):
                final = last_block and di == DM - 1
                acc = accs[di]
                if not final:
                    for i, j in ACT_TAPS:
                        nc.vector.tensor_max(acc[:], acc[:], atmps[(di, i, j)][:])
                    ring = nc.scalar if di == 0 else nc.sync
                    ring.dma_start(o_aps[di][:, r0 : r0 + rows, :], acc[:])
                else:
                    # final chain: last two merges in row halves, tap-outer so
                    # only the truly-last tmp's two half-merges sit behind
                    # ScalarE's final ACTIVATE in the in-order DVE queue
                    for i, j in ACT_TAPS_LAST[:-2]:
                        nc.vector.tensor_max(acc[:], acc[:], atmps[(di, i, j)][:])
                    hh = rows // 2
                    halves = ((0, hh, nc.scalar), (hh, rows, nc.sync))
                    i, j = ACT_TAPS_LAST[-2]
                    for h0, h1, _ in halves:
                        nc.vector.tensor_max(
                            acc[:, h0:h1, :], acc[:, h0:h1, :],
                            atmps[(di, i, j)][:, h0:h1, :],
                        )
                    i, j = ACT_TAPS_LAST[-1]
                    for h0, h1, ring in halves:
                        nc.vector.tensor_max(
                            acc[:, h0:h1, :], acc[:, h0:h1, :],
                            atmps[(di, i, j)][:, h0:h1, :],
                        )
                        ring.dma_start(
                            o_aps[di][:, r0 + h0 : r0 + h1, :], acc[:, h0:h1, :]
                        )

    _prune_redundant_waits(nc)
    _legalize_waits(nc)
    _CACHED[key] = nc
    return nc


def _host_prep(x: np.ndarray, kern: np.ndarray, dt_name: str):
    np_dt = {"bf16": BF16, "f16": np.float16, "f32": np.float32}[dt_name]
    pad = PAD_VAL_F16 if dt_name == "f16" else np.float32(-1e30)
    xp = np.full((B, H + 2, W + 2, C), pad, np.float32)
    xp[:, 1 : H + 1, 1 : W + 1, :] = x
    xp_t = np.ascontiguousarray(xp.astype(np_dt).transpose(0, 3, 1, 2))

    ktap = np.transpose(kern.astype(np.float32), (2, 3, 0, 1)).reshape(C, DM * 9)
    ktab = np.ascontiguousarray(np.tile(ktap, (G, 1)))

    in_maps = []
    for core in range(N_CORES):
        b, g2 = core // 2, core % 2
        xa = np.empty((128, XROWS, WP), np_dt)
        for g in range(G):
            r0 = g2 * 128 + g * HG
            xa[g * C : (g + 1) * C] = xp_t[b, :, r0 : r0 + XROWS, :]
        in_maps.append({"x": xa, "ktab": ktab})
    return in_maps


def _assemble(results):
    out = np.empty((B, H, W, DM * C), np.float32)
    for core in range(N_CORES):
        b, g2 = core // 2, core % 2
        for di in range(DM):
            o = np.asarray(results[core][f"o{di}"]).astype(np.float32)
            o4 = o.reshape(G, C, HG, W)
            for g in range(G):
                h0 = g2 * 128 + g * HG
                out[b, h0 : h0 + HG, :, di * C : (di + 1) * C] = o4[g].transpose(
                    1, 2, 0
                )
    return out


def run(x, kern, trace=False):
    nc = _build_bass(KERNEL_DT)
    in_maps = _host_prep(np.asarray(x, np.float32), np.asarray(kern, np.float32),
                         KERNEL_DT)
    r = run_bass_kernel_spmd(nc, in_maps, list(range(N_CORES)), trace=trace)
    return _assemble(r.results), r


def kernel(x, kernel):
    out, _ = run(x, kernel)
    return out


# revision 4
# speedup vs baseline: 1.0196x; 1.0196x over previous
"""Trainium2 Bass kernel v2 for depthwise morphological dilation (max-plus).

Same math/layout as kernel.py baseline; scheduling changes:
  - dual-ring DMA: head (block-0 x) split across qSync + qScalar; per-di
    output DMAs alternate rings; final output split across both.
  - redundant monotone sem-waits pruned before legalization (the baseline
    carried ~75 EventSemaphore instrs on ScalarE ~ 20us of issue time).
  - DVE emission interleaves the two di chains per block so DVE never
    idles waiting on ACT tmp production.
"""

import os
import sys

if "jax" not in sys.modules and os.environ.get("JAX_PLATFORMS") == "cpu":
    os.environ["JAX_PLATFORMS"] = ""

for _p in ("/opt/trn_rl_repo",):
    if _p not in sys.path:
        sys.path.insert(0, _p)

import ml_dtypes
import numpy as np

from concourse import mybir
import concourse.bass as bass
import concourse.tile as tile
from concourse.bass_utils import run_bass_kernel_spmd

BF16 = ml_dtypes.bfloat16

B, H, W, C = 4, 256, 256, 64
KH, KW, DM = 3, 3, 2
N_CORES = 8
PAD_VAL_F16 = np.float32(-60000.0)

G = 2
HG = 64
WP = W + 2
XROWS = HG + 2

KERNEL_DT = os.environ.get("KERNEL_DT", "f16")

# per-(block,di) tap assignment: init + 2 adds on DVE (all j-even, keeps
# 4x tensor_scalar), 6 adds on ACT (includes all j=1 taps).
INIT_TAP = (1, 0)
ACT_TAPS = [(0, 1), (1, 1), (2, 1), (0, 0), (2, 2), (0, 2)]
DVE_TAPS = [(2, 0), (1, 2)]
# the final (block,di) chain runs one more tap on DVE so its tail isn't
# gated on ScalarE's very last ACTIVATE
ACT_TAPS_LAST = [(0, 1), (1, 1), (2, 1), (0, 0), (2, 2)]
DVE_TAPS_LAST = [(2, 0), (1, 2), (0, 2)]

BLOCKS = [(0, 32), (32, 32)]

_CACHED = {}


def _prune_redundant_waits(nc):
    """Drop sem-ge-imm waits already implied by an earlier wait (same or
    higher threshold, same semaphore) on the same engine queue. Only for
    semaphores that are exclusively inc-updated (Tile's dependency
    counters); barrier sems (dec'd / eq-waited) are left alone."""
    unsafe = set()
    for fn in nc.m.functions:
        for blk in fn.blocks:
            for ins in blk.instructions:
                si = ins.sync_info
                if not si:
                    continue
                for u in si.on_update or []:
                    if u.update_mode != "sem-inc":
                        unsafe.add(u.id)
                for w in si.on_wait or []:
                    if w.wait_mode != "sem-ge-imm" or w.wait_reg is not None:
                        unsafe.add(w.id)
    n_drop = 0
    for fn in nc.m.functions:
        for blk in fn.blocks:
            best = {}  # (engine, sem_id) -> max threshold enforced
            for ins in blk.instructions:
                si = ins.sync_info
                if not si or not si.on_wait:
                    continue
                eng = str(ins.engine)
                keep = []
                for w in si.on_wait:
                    if (
                        w.wait_mode == "sem-ge-imm"
                        and w.wait_reg is None
                        and w.id not in unsafe
                    ):
                        k = (eng, w.id)
                        if best.get(k, -1) >= w.wait_value:
                            n_drop += 1
                            continue
                        best[k] = w.wait_value
                    keep.append(w)
                si.on_wait = keep
    return n_drop


def _legalize_waits(nc):
    """Split multi-wait instructions (TPB ISA: one sem-wait per regular
    instruction; EventSemaphore holds 2)."""
    n_extra = 0
    for fn in nc.m.functions:
        for blk in fn.blocks:
            insts = blk.instructions
            new_list = []
            for ins in insts:
                si = ins.sync_info
                waits = list(si.on_wait) if (si and si.on_wait) else []
                if len(waits) > 1 and ins.opcode != "EventSemaphore":
                    keep, extra = waits[-1], waits[:-1]
                    for k in range(0, len(extra), 2):
                        es = mybir.InstEventSemaphore(
                            name=f"legalw_{ins.name}_{k}", ins=[], outs=[]
                        )
                        es.engine = ins.engine
                        es.sync_info = mybir.SyncInfo(
                            on_wait=extra[k : k + 2], on_update=[]
                        )
                        new_list.append(es)
                        n_extra += 1
                    si.on_wait = [keep]
                new_list.append(ins)
            insts[:] = new_list
    return n_extra


def _build_bass(dt_name: str):
    key = dt_name
    if key in _CACHED:
        return _CACHED[key]
    dt = {
        "bf16": mybir.dt.bfloat16,
        "f16": mybir.dt.float16,
        "f32": mybir.dt.float32,
    }[dt_name]

    nc = bass.Bass(
        "TRN2",
        target_bir_lowering=False,
        debug=False,
        num_devices=N_CORES,
    )
    x_ap = nc.dram_tensor("x", [128, XROWS, WP], dt, kind="ExternalInput").ap()
    kt_ap = nc.dram_tensor(
        "ktab", [128, KH * KW * DM], mybir.dt.float32, kind="ExternalInput"
    ).ap()
    o_aps = [
        nc.dram_tensor(f"o{di}", [128, HG, W], dt, kind="ExternalOutput").ap()
        for di in range(DM)
    ]

    from contextlib import ExitStack

    with tile.TileContext(nc) as tc, ExitStack() as ctx:
        kpool = ctx.enter_context(tc.tile_pool(name="kp", bufs=1))
        xpool = ctx.enter_context(tc.tile_pool(name="xp", bufs=2))
        tpool = ctx.enter_context(tc.tile_pool(name="tp", bufs=6))       # ACT tmps
        vpool = ctx.enter_context(tc.tile_pool(name="vp", bufs=2))       # DVE tmps
        apool = ctx.enter_context(tc.tile_pool(name="acp", bufs=2))

        ktile = kpool.tile([128, KH * KW * DM], mybir.dt.float32, name="ktile")
        nc.sync.dma_start(ktile[:], kt_ap[:])

        # input blocks spread across both DMA rings: the tiny block-0 lands
        # first on qSync so compute starts early; block-1 rides qScalar in
        # parallel (ScalarE is idle at the head)
        xbs = []
        for bi, (r0, rows) in enumerate(BLOCKS):
            xb = xpool.tile([128, rows + 2, WP], dt, name="xb", tag="xb")
            if bi == 0:
                half = (rows + 2) // 2
                nc.sync.dma_start(xb[:, :half, :], x_ap[:, r0 : r0 + half, :])
                nc.scalar.dma_start(
                    xb[:, half : rows + 2, :],
                    x_ap[:, r0 + half : r0 + rows + 2, :],
                )
            else:
                nc.sync.dma_start(xb[:], x_ap[:, r0 : r0 + rows + 2, :])
            xbs.append(xb)

        # ACT warmup: zero-dependency ACTIVATE so the activation-table load
        # overlaps the input DMA.
        warm = kpool.tile([128, 1], dt, name="warm")
        nc.gpsimd.memset(warm[:], 0.0)
        nc.scalar.add(warm[:], warm[:], 0.0)

        def kvec(i, j, di):
            t = di * 9 + i * 3 + j
            return ktile[:, t : t + 1]

        n_blocks = len(BLOCKS)
        for blk, (r0, rows) in enumerate(BLOCKS):
            xb = xbs[blk]
            last_block = blk == n_blocks - 1

            def src(i, j):
                return xb[:, i : i + rows, j : j + W]

            # ---- ACT: tmps for this block (di0 then di1) ----
            atmps = {}
            for di in range(DM):
                final = last_block and di == DM - 1
                for i, j in ACT_TAPS_LAST if final else ACT_TAPS:
                    tmpa = tpool.tile([128, rows, W], dt, name="tmpa", tag="tmp")
                    nc.scalar.add(tmpa[:], src(i, j), kvec(i, j, di))
                    atmps[(di, i, j)] = tmpa

            # ---- DVE: per-di front (init + own adds + own maxes) ----
            accs = {}
            for di in range(DM):
                final = last_block and di == DM - 1
                acc = apool.tile([128, rows, W], dt, name="acc", tag="acc")
                i, j = INIT_TAP
                nc.vector.tensor_scalar_add(acc[:], src(i, j), kvec(i, j, di))
                accs[di] = acc
                dtmps = []
                for i, j in DVE_TAPS_LAST if final else DVE_TAPS:
                    tv = vpool.tile([128, rows, W], dt, name="tmpv", tag="vtmp")
                    nc.vector.tensor_scalar_add(tv[:], src(i, j), kvec(i, j, di))
                    dtmps.append(tv)
                for tv in dtmps:
                    nc.vector.tensor_max(acc[:], acc[:], tv[:])
            # merge ACT tmps in production order per di
            for di in range(DM):
                final = last_block and di == DM - 1
                acc = accs[di]
                if not final:
                    for i, j in ACT_TAPS:
                        nc.vector.tensor_max(acc[:], acc[:], atmps[(di, i, j)][:])
                    ring = nc.gpsimd if di == 0 else nc.sync
                    ring.dma_start(o_aps[di][:, r0 : r0 + rows, :], acc[:])
                else:
                    # final chain: last two merges in row halves, tap-outer so
                    # only the truly-last tmp's two half-merges sit behind
                    # ScalarE's final ACTIVATE in the in-order DVE queue
                    for i, j in ACT_TAPS_LAST[:-2]:
                        nc.vector.tensor_max(acc[:], acc[:], atmps[(di, i, j)][:])
                    hh = rows // 2
                    halves = ((0, hh, nc.scalar), (hh, rows, nc.sync))
                    i, j = ACT_TAPS_LAST[-2]
                    for h0, h1, _ in halves:
                        nc.vector.tensor_max(
                            acc[:, h0:h1, :], acc[:, h0:h1, :],
                            atmps[(di, i, j)][:, h0:h1, :],
                        )
                    i, j = ACT_TAPS_LAST[-1]
                    for h0, h1, ring in halves:
                        nc.vector.tensor_max(
                            acc[:, h0:h1, :], acc[:, h0:h1, :],
                            atmps[(di, i, j)][:, h0:h1, :],
                        )
                        ring.dma_start(
                            o_aps[di][:, r0 + h0 : r0 + h1, :], acc[:, h0:h1, :]
                        )

    _prune_redundant_waits(nc)
    _legalize_waits(nc)
    _CACHED[key] = nc
    return nc


def _host_prep(x: np.ndarray, kern: np.ndarray, dt_name: str):
    np_dt = {"bf16": BF16, "f16": np.float16, "f32": np.float32}[dt_name]
    pad = PAD_VAL_F16 if dt_name == "f16" else np.float32(-1e30)
    xp = np.full((B, H + 2, W + 2, C), pad, np.float32)
    xp[:, 1 : H + 1, 1 : W + 1, :] = x
    xp_t = np.ascontiguousarray(xp.astype(np_dt).transpose(0, 3, 1, 2))

    ktap = np.transpose(kern.astype(np.float32), (2, 3, 0, 1)).reshape(C, DM * 9)
    ktab = np.ascontiguousarray(np.tile(ktap, (G, 1)))

    in_maps = []
    for core in range(N_CORES):
        b, g2 = core // 2, core % 2
        xa = np.empty((128, XROWS, WP), np_dt)
        for g in range(G):
            r0 = g2 * 128 + g * HG
            xa[g * C : (g + 1) * C] = xp_t[b, :, r0 : r0 + XROWS, :]
        in_maps.append({"x": xa, "ktab": ktab})
    return in_maps


def _assemble(results):
    out = np.empty((B, H, W, DM * C), np.float32)
    for core in range(N_CORES):
        b, g2 = core // 2, core % 2
        for di in range(DM):
            o = np.asarray(results[core][f"o{di}"]).astype(np.float32)
            o4 = o.reshape(G, C, HG, W)
            for g in range(G):
                h0 = g2 * 128 + g * HG
                out[b, h0 : h0 + HG, :, di * C : (di + 1) * C] = o4[g].transpose(
                    1, 2, 0
                )
    return out


def run(x, kern, trace=False):
    nc = _build_bass(KERNEL_DT)
    in_maps = _host_prep(np.asarray(x, np.float32), np.asarray(kern, np.float32),
                         KERNEL_DT)
    r = run_bass_kernel_spmd(nc, in_maps, list(range(N_CORES)), trace=trace)
    return _assemble(r.results), r


def kernel(x, kernel):
    out, _ = run(x, kernel)
    return out


# revision 5
# speedup vs baseline: 1.0228x; 1.0031x over previous
"""Trainium2 Bass kernel v2 for depthwise morphological dilation (max-plus).

Same math/layout as kernel.py baseline; scheduling changes:
  - dual-ring DMA: head (block-0 x) split across qSync + qScalar; per-di
    output DMAs alternate rings; final output split across both.
  - redundant monotone sem-waits pruned before legalization (the baseline
    carried ~75 EventSemaphore instrs on ScalarE ~ 20us of issue time).
  - DVE emission interleaves the two di chains per block so DVE never
    idles waiting on ACT tmp production.
"""

import os
import sys

if "jax" not in sys.modules and os.environ.get("JAX_PLATFORMS") == "cpu":
    os.environ["JAX_PLATFORMS"] = ""

for _p in ("/opt/trn_rl_repo",):
    if _p not in sys.path:
        sys.path.insert(0, _p)

import ml_dtypes
import numpy as np

from concourse import mybir
import concourse.bass as bass
import concourse.tile as tile
from concourse.bass_utils import run_bass_kernel_spmd

BF16 = ml_dtypes.bfloat16

B, H, W, C = 4, 256, 256, 64
KH, KW, DM = 3, 3, 2
N_CORES = 8
PAD_VAL_F16 = np.float32(-60000.0)

G = 2
HG = 64
WP = W + 2
XROWS = HG + 2

KERNEL_DT = os.environ.get("KERNEL_DT", "f16")

# per-(block,di) tap assignment: init + 2 adds on DVE (all j-even, keeps
# 4x tensor_scalar), 6 adds on ACT (includes all j=1 taps).
INIT_TAP = (1, 0)
ACT_TAPS = [(0, 1), (1, 1), (2, 1), (0, 0), (2, 2), (0, 2)]
DVE_TAPS = [(2, 0), (1, 2)]
# the final (block,di) chain runs one more tap on DVE so its tail isn't
# gated on ScalarE's very last ACTIVATE
ACT_TAPS_LAST = [(0, 1), (1, 1), (2, 1), (0, 0), (2, 2)]
DVE_TAPS_LAST = [(2, 0), (1, 2), (0, 2)]

BLOCKS = [(0, 32), (32, 32)]

_CACHED = {}


def _prune_redundant_waits(nc):
    """Drop sem-ge-imm waits already implied by an earlier wait (same or
    higher threshold, same semaphore) on the same engine queue. Only for
    semaphores that are exclusively inc-updated (Tile's dependency
    counters); barrier sems (dec'd / eq-waited) are left alone."""
    unsafe = set()
    for fn in nc.m.functions:
        for blk in fn.blocks:
            for ins in blk.instructions:
                si = ins.sync_info
                if not si:
                    continue
                for u in si.on_update or []:
                    if u.update_mode != "sem-inc":
                        unsafe.add(u.id)
                for w in si.on_wait or []:
                    if w.wait_mode != "sem-ge-imm" or w.wait_reg is not None:
                        unsafe.add(w.id)
    n_drop = 0
    for fn in nc.m.functions:
        for blk in fn.blocks:
            best = {}  # (engine, sem_id) -> max threshold enforced
            for ins in blk.instructions:
                si = ins.sync_info
                if not si or not si.on_wait:
                    continue
                eng = str(ins.engine)
                keep = []
                for w in si.on_wait:
                    if (
                        w.wait_mode == "sem-ge-imm"
                        and w.wait_reg is None
                        and w.id not in unsafe
                    ):
                        k = (eng, w.id)
                        if best.get(k, -1) >= w.wait_value:
                            n_drop += 1
                            continue
                        best[k] = w.wait_value
                    keep.append(w)
                si.on_wait = keep
    return n_drop


def _legalize_waits(nc):
    """Split multi-wait instructions (TPB ISA: one sem-wait per regular
    instruction; EventSemaphore holds 2)."""
    n_extra = 0
    for fn in nc.m.functions:
        for blk in fn.blocks:
            insts = blk.instructions
            new_list = []
            for ins in insts:
                si = ins.sync_info
                waits = list(si.on_wait) if (si and si.on_wait) else []
                if len(waits) > 1 and ins.opcode != "EventSemaphore":
                    keep, extra = waits[-1], waits[:-1]
                    for k in range(0, len(extra), 2):
                        es = mybir.InstEventSemaphore(
                            name=f"legalw_{ins.name}_{k}", ins=[], outs=[]
                        )
                        es.engine = ins.engine
                        es.sync_info = mybir.SyncInfo(
                            on_wait=extra[k : k + 2], on_update=[]
                        )
                        new_list.append(es)
                        n_extra += 1
                    si.on_wait = [keep]
                new_list.append(ins)
            insts[:] = new_list
    return n_extra


def _build_bass(dt_name: str):
    key = dt_name
    if key in _CACHED:
        return _CACHED[key]
    dt = {
        "bf16": mybir.dt.bfloat16,
        "f16": mybir.dt.float16,
        "f32": mybir.dt.float32,
    }[dt_name]

    nc = bass.Bass(
        "TRN2",
        target_bir_lowering=False,
        debug=False,
        num_devices=N_CORES,
    )
    x_ap = nc.dram_tensor("x", [128, XROWS, WP], dt, kind="ExternalInput").ap()
    kt_ap = nc.dram_tensor(
        "ktab", [128, KH * KW * DM], mybir.dt.float32, kind="ExternalInput"
    ).ap()
    o_aps = [
        nc.dram_tensor(f"o{di}", [128, HG, W], dt, kind="ExternalOutput").ap()
        for di in range(DM)
    ]

    from contextlib import ExitStack

    with tile.TileContext(nc) as tc, ExitStack() as ctx:
        kpool = ctx.enter_context(tc.tile_pool(name="kp", bufs=1))
        xpool = ctx.enter_context(tc.tile_pool(name="xp", bufs=2))
        tpool = ctx.enter_context(tc.tile_pool(name="tp", bufs=6))       # ACT tmps
        vpool = ctx.enter_context(tc.tile_pool(name="vp", bufs=2))       # DVE tmps
        apool = ctx.enter_context(tc.tile_pool(name="acp", bufs=2))

        ktile = kpool.tile([128, KH * KW * DM], mybir.dt.float32, name="ktile")
        nc.sync.dma_start(ktile[:], kt_ap[:])

        # input blocks spread across both DMA rings: the tiny block-0 lands
        # first on qSync so compute starts early; block-1 rides qScalar in
        # parallel (ScalarE is idle at the head)
        xbs = []
        for bi, (r0, rows) in enumerate(BLOCKS):
            xb = xpool.tile([128, rows + 2, WP], dt, name="xb", tag="xb")
            if bi == 0:
                half = (rows + 2) // 2
                nc.sync.dma_start(xb[:, :half, :], x_ap[:, r0 : r0 + half, :])
                nc.scalar.dma_start(
                    xb[:, half : rows + 2, :],
                    x_ap[:, r0 + half : r0 + rows + 2, :],
                )
            else:
                nc.sync.dma_start(xb[:], x_ap[:, r0 : r0 + rows + 2, :])
            xbs.append(xb)

        # ACT warmup: zero-dependency ACTIVATE so the activation-table load
        # overlaps the input DMA.
        warm = kpool.tile([128, 1], dt, name="warm")
        nc.gpsimd.memset(warm[:], 0.0)
        nc.scalar.add(warm[:], warm[:], 0.0)

        def kvec(i, j, di):
            t = di * 9 + i * 3 + j
            return ktile[:, t : t + 1]

        n_blocks = len(BLOCKS)
        for blk, (r0, rows) in enumerate(BLOCKS):
            xb = xbs[blk]
            last_block = blk == n_blocks - 1

            def src(i, j):
                return xb[:, i : i + rows, j : j + W]

            # ---- ACT: tmps for this block (di0 then di1) ----
            atmps = {}
            for di in range(DM):
                final = last_block and di == DM - 1
                for i, j in ACT_TAPS_LAST if final else ACT_TAPS:
                    tmpa = tpool.tile([128, rows, W], dt, name="tmpa", tag="tmp")
                    nc.scalar.add(tmpa[:], src(i, j), kvec(i, j, di))
                    atmps[(di, i, j)] = tmpa

            # ---- DVE: per-di front (init + own adds + own maxes) ----
            accs = {}
            for di in range(DM):
                final = last_block and di == DM - 1
                acc = apool.tile([128, rows, W], dt, name="acc", tag="acc")
                i, j = INIT_TAP
                nc.vector.tensor_scalar_add(acc[:], src(i, j), kvec(i, j, di))
                accs[di] = acc
                dtmps = []
                for i, j in DVE_TAPS_LAST if final else DVE_TAPS:
                    tv = vpool.tile([128, rows, W], dt, name="tmpv", tag="vtmp")
                    nc.vector.tensor_scalar_add(tv[:], src(i, j), kvec(i, j, di))
                    dtmps.append(tv)
                for tv in dtmps:
                    nc.vector.tensor_max(acc[:], acc[:], tv[:])
            # merge ACT tmps in production order per di
            for di in range(DM):
                final = last_block and di == DM - 1
                acc = accs[di]
                if not final:
                    for i, j in ACT_TAPS:
                        nc.vector.tensor_max(acc[:], acc[:], atmps[(di, i, j)][:])
                    ring = nc.gpsimd if di == 0 else nc.sync
                    ring.dma_start(o_aps[di][:, r0 : r0 + rows, :], acc[:])
                else:
                    # final chain: last two merges in row halves, tap-outer so
                    # only the truly-last tmp's two half-merges sit behind
                    # ScalarE's final ACTIVATE in the in-order DVE queue
                    for i, j in ACT_TAPS_LAST[:-2]:
                        nc.vector.tensor_max(acc[:], acc[:], atmps[(di, i, j)][:])
                    th = rows // 3
                    halves = (
                        (0, th, nc.scalar),
                        (th, 2 * th, nc.gpsimd),
                        (2 * th, rows, nc.sync),
                    )
                    i, j = ACT_TAPS_LAST[-2]
                    for h0, h1, _ in halves:
                        nc.vector.tensor_max(
                            acc[:, h0:h1, :], acc[:, h0:h1, :],
                            atmps[(di, i, j)][:, h0:h1, :],
                        )
                    i, j = ACT_TAPS_LAST[-1]
                    for h0, h1, ring in halves:
                        nc.vector.tensor_max(
                            acc[:, h0:h1, :], acc[:, h0:h1, :],
                            atmps[(di, i, j)][:, h0:h1, :],
                        )
                        ring.dma_start(
                            o_aps[di][:, r0 + h0 : r0 + h1, :], acc[:, h0:h1, :]
                        )

    _prune_redundant_waits(nc)
    _legalize_waits(nc)
    _CACHED[key] = nc
    return nc


def _host_prep(x: np.ndarray, kern: np.ndarray, dt_name: str):
    np_dt = {"bf16": BF16, "f16": np.float16, "f32": np.float32}[dt_name]
    pad = PAD_VAL_F16 if dt_name == "f16" else np.float32(-1e30)
    xp = np.full((B, H + 2, W + 2, C), pad, np.float32)
    xp[:, 1 : H + 1, 1 : W + 1, :] = x
    xp_t = np.ascontiguousarray(xp.astype(np_dt).transpose(0, 3, 1, 2))

    ktap = np.transpose(kern.astype(np.float32), (2, 3, 0, 1)).reshape(C, DM * 9)
    ktab = np.ascontiguousarray(np.tile(ktap, (G, 1)))

    in_maps = []
    for core in range(N_CORES):
        b, g2 = core // 2, core % 2
        xa = np.empty((128, XROWS, WP), np_dt)
        for g in range(G):
            r0 = g2 * 128 + g * HG
            xa[g * C : (g + 1) * C] = xp_t[b, :, r0 : r0 + XROWS, :]
        in_maps.append({"x": xa, "ktab": ktab})
    return in_maps


def _assemble(results):
    out = np.empty((B, H, W, DM * C), np.float32)
    for core in range(N_CORES):
        b, g2 = core // 2, core % 2
        for di in range(DM):
            o = np.asarray(results[core][f"o{di}"]).astype(np.float32)
            o4 = o.reshape(G, C, HG, W)
            for g in range(G):
                h0 = g2 * 128 + g * HG
                out[b, h0 : h0 + HG, :, di * C : (di + 1) * C] = o4[g].transpose(
                    1, 2, 0
                )
    return out


def run(x, kern, trace=False):
    nc = _build_bass(KERNEL_DT)
    in_maps = _host_prep(np.asarray(x, np.float32), np.asarray(kern, np.float32),
                         KERNEL_DT)
    r = run_bass_kernel_spmd(nc, in_maps, list(range(N_CORES)), trace=trace)
    return _assemble(r.results), r


def kernel(x, kernel):
    out, _ = run(x, kernel)
    return out
